# revision 1
# baseline (speedup 1.0000x reference)
"""Pipeline T: spatial-layout depthwise (banded matmuls) + PE transpose + pointwise.

Per core (4 batches):
  x host-prepped to [b, h, ci, 114w] (W zero-padded, h-major) -> SBUF
  x_sb [112 h, (96 ci, 114 w)]   (two half-slabs of 48 ci each)
  DW:  per 4-ci group: 3 accumulated matmuls, lhsT = band B_v [112h,112i]
       (B_v[h,i] = k3[h-i+1, v]), rhs = x_sb[:, (4ci,114), cols v..v+112]
       -> PSUM [112 i, (4 ci, 112 j)]  (H-taps via band, W-taps via rhs shift)
  Q_sb [112 i, (96 ci, 112 j)]
  TR:  per j: transpose(lhsT=Q_sb[:, ci-stride gather @ j], identity)
       -> PSUM [96 ci, 112 i]; 4 j per bank; evac -> Qt_sb [96, j*112+i]
  PW:  per 4-row chunk: mt0 M=128, mt1 M=64 (even/odd chunks packed into
       one [128,448] PSUM tile via tile_position=(0,64)) -> y
"""

import numpy as np

from concourse import bacc, mybir
from concourse import tile
from concourse.bass_utils import run_bass_kernel_spmd

F32 = mybir.dt.float32
F32R = mybir.dt.float32r

B, C_IN, C_OUT, H, W = 32, 96, 192, 112, 112
N_CORES = 8
B_PER = B // N_CORES
WP = W + 2                      # 114 padded width
CI_G = 4                        # ci images per DW matmul group
N_G = C_IN // CI_G              # 24 groups per batch
RPC = 4                         # output rows per PW chunk -> N = 448
N_CHUNKS = H // RPC             # 28
CPB = 4                         # PW chunks per out-DMA block (16 rows)
N_BLOCKS = N_CHUNKS // CPB      # 7

_NC = None
LAST_RESULTS = None


def _build():
    nc = bacc.Bacc("TRN2", target_bir_lowering=False, debug=False,
                   num_devices=N_CORES)

    # x: [b, h, ci, wp]  (host pre-transposed + W-padded)
    x_d = nc.dram_tensor("x", [B_PER, H, C_IN, WP], F32R, kind="ExternalInput")
    band_d = nc.dram_tensor("band", [H, 3, H], F32R, kind="ExternalInput")
    ident_d = nc.dram_tensor("ident", [H, H], F32R, kind="ExternalInput")
    wpcT_d = nc.dram_tensor("wpcT", [C_IN, C_OUT], F32R, kind="ExternalInput")
    y_d = nc.dram_tensor("y", [B_PER, C_OUT, H, W], F32, kind="ExternalOutput")

    HALF = C_IN // 2            # 48 ci per x half-slab

    with tile.TileContext(nc) as tc:
        with (
            tc.tile_pool(name="consts", bufs=1) as consts,
            tc.tile_pool(name="xin", bufs=3) as xin,
            tc.tile_pool(name="qsb", bufs=1) as qsbp,
            tc.tile_pool(name="qtb", bufs=1) as qtbp,
            tc.tile_pool(name="ys", bufs=2) as ysp,
            tc.tile_pool(name="qp", bufs=2, space="PSUM") as qpp,
            tc.tile_pool(name="tp", bufs=2, space="PSUM") as tpp,
            tc.tile_pool(name="yp0", bufs=4, space="PSUM") as yp0p,
        ):
            band_sb = consts.tile([H, 3, H], F32R)
            nc.sync.dma_start(band_sb[:], band_d[:])
            ident_sb = consts.tile([H, H], F32R)
            nc.sync.dma_start(ident_sb[:], ident_d[:])
            wpc_sb = consts.tile([C_IN, C_OUT], F32R)
            nc.sync.dma_start(wpc_sb[:], wpcT_d[:])

            copy_ctr = 0

            for b in range(B_PER):
                # -------- load x half-slabs: [112 h, 48 ci, 114 w] --------
                xh = []
                for hf in range(2):
                    xt = xin.tile([H, HALF, WP], F32R, name=f"xh{hf}",
                                  tag="xh")
                    nc.gpsimd.dma_start(
                        xt[:], x_d[b, :, hf * HALF:(hf + 1) * HALF, :])
                    xh.append(xt)

                # -------- depthwise: banded matmuls --------
                # Q_sb [112 i, (96 ci, 112 j)]
                qsb = qsbp.tile([H, C_IN, W], F32R)
                for g3 in range((N_G + 1) // 2):
                    gs = [g for g in (2 * g3, 2 * g3 + 1) if g < N_G]
                    qps = []
                    for g in gs:
                        qp = qpp.tile([H, CI_G, W], F32, name="qp", tag="qp")
                        qps.append(qp)
                    for v in range(3):
                        for qp, g in zip(qps, gs):
                            hf, gg = divmod(g, N_G // 2)
                            rhs = xh[hf][:, gg * CI_G:(gg + 1) * CI_G,
                                         v:v + W]
                            nc.tensor.matmul(
                                qp[:], band_sb[:, v, :], rhs,
                                start=(v == 0), stop=(v == 2),
                            )
                    for qp, g in zip(qps, gs):
                        dst = qsb[:, g * CI_G:(g + 1) * CI_G, :]
                        if copy_ctr % 2 == 0:
                            nc.scalar.copy(dst, qp[:])
                        else:
                            nc.vector.tensor_copy(dst, qp[:])
                        copy_ctr += 1

                # -------- transpose: i<->ci per j column --------
                # Qt_sb [96 ci, (112 i, 112 j)]  addr = i*112 + j
                qtb = qtbp.tile([C_IN, H, W], F32R)
                for j4 in range(W // 4):
                    tp = tpp.tile([C_IN, 4, H], F32R, name="tp", tag="tp")
                    for jj in range(4):
                        j = 4 * j4 + jj
                        lhsT = qsb[:, :, j]          # [112 i, 96 ci] stride W
                        nc.tensor.transpose(
                            tp[:, jj, :], lhsT, ident_sb[:])
                    dst = qtb[:, :, 4 * j4:4 * j4 + 4]\
                        .rearrange("c i j -> c j i")
                    if copy_ctr % 2 == 0:
                        nc.scalar.copy(dst, tp[:])
                    else:
                        nc.vector.tensor_copy(dst, tp[:])
                    copy_ctr += 1

                # -------- pointwise: two 96-output halves --------
                # mt-major within each CPB-chunk block: 1 LDW per CPB MMs
                for blk in range(N_BLOCKS):
                    ys = [None, None]
                    for mt in range(2):
                        ys[mt] = ysp.tile([96, CPB, RPC, W], F32,
                                          name=f"ys{mt}", tag=f"ys{mt}")
                        yps = []
                        for slot in range(CPB):
                            i0 = (blk * CPB + slot) * RPC
                            rhs = qtb[:, i0:i0 + RPC, :]
                            yp = yp0p.tile([96, RPC, W], F32, name="yp",
                                           tag="yp")
                            nc.tensor.matmul(
                                yp[:], wpc_sb[:, mt * 96:(mt + 1) * 96],
                                rhs, start=True, stop=True)
                            yps.append(yp)
                        for slot, yp in enumerate(yps):
                            dst = ys[mt][:, slot, :, :]
                            if copy_ctr % 2 == 0:
                                nc.scalar.copy(dst, yp[:])
                            else:
                                nc.vector.tensor_copy(dst, yp[:])
                            copy_ctr += 1
                        r0 = blk * CPB * RPC
                        nc.sync.dma_start(
                            y_d[b, mt * 96:(mt + 1) * 96,
                                r0:r0 + CPB * RPC, :],
                            ys[mt][:].rearrange("p c r w -> p (c r) w"),
                        )

    nc.compile()
    return nc


def _prep_inputs(x, w_pc, w_dc):
    x = np.asarray(x, dtype=np.float32)
    k3 = np.asarray(w_dc, dtype=np.float32).reshape(3, 3)
    Wm = np.asarray(w_pc, dtype=np.float32).reshape(C_OUT, C_IN)

    # [b, h, ci, 114]: transpose + W-pad
    xp = np.zeros((B, H, C_IN, WP), dtype=np.float32)
    xp[:, :, :, 1:1 + W] = x.transpose(0, 2, 1, 3)

    # band[h, v, i] = k3[h - i + 1, v]
    band = np.zeros((H, 3, H), dtype=np.float32)
    hh, ii = np.meshgrid(np.arange(H), np.arange(H), indexing="ij")
    u = hh - ii + 1
    m = (u >= 0) & (u < 3)
    for v in range(3):
        bv = np.zeros((H, H), dtype=np.float32)
        bv[m] = k3[u[m], v]
        band[:, v, :] = bv

    ident = np.eye(H, dtype=np.float32)
    wpcT = np.ascontiguousarray(Wm.T)
    return xp, band, ident, wpcT


def kernel(x, w_pc, w_dc, _trace=False):
    global _NC, LAST_RESULTS
    if _NC is None:
        _NC = _build()

    xp, band, ident, wpcT = _prep_inputs(x, w_pc, w_dc)
    in_maps = [
        {"x": np.ascontiguousarray(xp[i * B_PER:(i + 1) * B_PER]),
         "band": band, "ident": ident, "wpcT": wpcT}
        for i in range(N_CORES)
    ]
    res = run_bass_kernel_spmd(_NC, in_maps, list(range(N_CORES)),
                               trace=_trace)
    LAST_RESULTS = res
    y = np.concatenate([res.results[i]["y"] for i in range(N_CORES)], axis=0)
    return np.asarray(y, dtype=np.float32)



# revision 8
# speedup vs baseline: 1.3547x; 1.3547x over previous
"""Pipeline X2: banded-matmul depthwise + DMA-xbar transpose + pointwise, bf16.

All free-dim walks are contiguous-inner (the PE moving operand and ACT/DVE
copies need >=16B inner runs; strided 2B walks run ~5x slower).

Per core (4 batches):
  x host-prepped to [b, h, 114w, 96ci] bf16 (W zero-padded, ci INNER)
  DW:  per 8-j tile (2 PSUM banks, 4 j each): 3 taps x 2 sub-blocks,
       lhsT = band B_v [112h, 128i] (i-padded for FWL),
       rhs = xt[:, j0+4t+v : +4, :] [112h, (4j, 96ci)] N=384
       -> PSUM qp [128i, 8j, 128cpad] (each 4j-slice = one bank)
       evac -> qsb [112 i, 112 j, 128 cpad]   (contiguous both sides)
  TR:  one DMA xbar transpose per batch (off the PE): the HW xbar folds the
       flat free dim by 128, so qsb's inner dim is ci padded to 128:
       qsb [112, (j,c128)] -> qtb [128 ci, 112 j, 112 i] (parts 96..127 junk)
  PW:  per 4-j chunk: rhs = qtb[0:96, j0:j0+4, :] (native layout, i inner),
       co split {128, 64}: 2 matmuls N=448 -> ys staging [co, j, i]
  y stored [b, co, j, i]; host transposes to [b, co, i, j] and upcasts fp32.

Emission order DW(b), xbar(b), PW(b-1) keeps the PE fed while the xbar
transpose of batch b is in flight.
"""

import numpy as np
import ml_dtypes

from concourse import bacc, mybir
from concourse import tile
from concourse.bass_utils import run_bass_kernel_spmd

F32 = mybir.dt.float32
BF16 = mybir.dt.bfloat16

B, C_IN, C_OUT, H, W = 32, 96, 192, 112, 112
N_CORES = 8
B_PER = B // N_CORES
WP = W + 2                      # 114 padded width
IPAD = 128                      # band i-dim padded for FWL weight loads
CPAD = 128                      # qsb ci padded to the xbar fold width
JB = 4                          # j rows per DW matmul (N = 4*96 = 384)
JT = 8                          # j rows per DW PSUM tile (2 banks)
N_JT = W // JT                  # 14 DW tiles per batch
RPC = 4                         # j rows per PW chunk -> N = 448
N_CHUNKS = W // RPC             # 28
CPB = 7                         # PW chunks per out-DMA block (28 j rows)
N_BLOCKS = N_CHUNKS // CPB      # 4

_NC = None
LAST_RESULTS = None


def _build():
    nc = bacc.Bacc("TRN2", target_bir_lowering=False, debug=False,
                   num_devices=N_CORES)

    # x: [b, h, wp, ci]  (host pre-transposed + W-padded, ci inner, bf16)
    x_d = nc.dram_tensor("x", [B_PER, H, WP, C_IN], BF16, kind="ExternalInput")
    band_d = nc.dram_tensor("band", [H, 3, IPAD], BF16, kind="ExternalInput")
    wpcT_d = nc.dram_tensor("wpcT", [C_IN, C_OUT], BF16, kind="ExternalInput")
    # y stored transposed: [b, co, j, i]; host swaps back to [b, co, i, j]
    y_d = nc.dram_tensor("y", [B_PER, C_OUT, W, H], BF16,
                         kind="ExternalOutput")

    with tile.TileContext(nc) as tc:
        with (
            tc.tile_pool(name="consts", bufs=1) as consts,
            tc.tile_pool(name="xin", bufs=2) as xin,
            tc.tile_pool(name="qsb", bufs=2) as qsbp,
            tc.tile_pool(name="qtb", bufs=2) as qtbp,
            tc.tile_pool(name="ysa", bufs=2) as ysap,
            tc.tile_pool(name="ysb", bufs=2) as ysbp,
            tc.tile_pool(name="qp", bufs=4, space="PSUM") as qpp,
            tc.tile_pool(name="ypa", bufs=2, space="PSUM") as ypap,
            tc.tile_pool(name="ypb", bufs=2, space="PSUM") as ypbp,
        ):
            band_sb = consts.tile([H, 3, IPAD], BF16)
            nc.sync.dma_start(band_sb[:], band_d[:])
            wpc_sb = consts.tile([C_IN, C_OUT], BF16)
            nc.sync.dma_start(wpc_sb[:], wpcT_d[:])

            copy_ctr = 0
            qtbs = [None] * B_PER

            def emit_dw(b):
                nonlocal copy_ctr
                xt = xin.tile([H, WP, C_IN], BF16, name=f"xt{b}", tag="xt")
                nc.gpsimd.dma_start(xt[:], x_d[b])

                # qsb [112 i, 112 j, 128 cpad] (ci inner)
                qsb = qsbp.tile([H, W, CPAD], BF16, name=f"qsb{b}", tag="qsb")
                # zero the pad cols (the xbar reads them; overlaps DW compute
                # on the otherwise-idle gpsimd engine)
                nc.gpsimd.memset(qsb[:, :, C_IN:CPAD], 0)
                for jt in range(W // JB):
                    # one PSUM bank per 4-j block, contiguous [128, 384] out
                    qp = qpp.tile([IPAD, 512], F32, name="qp", tag="qp")
                    j0 = jt * JB
                    for v in range(3):
                        nc.tensor.matmul(
                            qp[:, 0:JB * C_IN],
                            band_sb[:, v, :],
                            xt[:, j0 + v:j0 + v + JB, :],
                            start=(v == 0), stop=(v == 2),
                        )
                    src = qp[0:H, 0:JB * C_IN]
                    dst = qsb[:, j0:j0 + JB, 0:C_IN]
                    if copy_ctr % 2 == 0:
                        nc.scalar.copy(dst, src)
                    else:
                        nc.vector.tensor_copy(dst, src)
                    copy_ctr += 1

                # DMA xbar transpose (all 16 DMA engines, off the PE):
                # [112 i, (j, c128)] -> [128 c, 112 j, 112 i]
                qtb = qtbp.tile([CPAD, W, H], BF16, name=f"qtb{b}", tag="qtb")
                nc.sync.dma_start(qtb[:], qsb[:].rearrange("i j c -> i (j c)"),
                                  transpose=True)
                qtbs[b] = qtb

            def emit_pw(b):
                nonlocal copy_ctr
                qtb = qtbs[b]
                for q in range(N_BLOCKS):
                    ysa = ysap.tile([128, CPB * RPC, H], BF16,
                                    name=f"ysa{b}_{q}", tag="ysa")
                    ysb = ysbp.tile([64, CPB * RPC, H], BF16,
                                    name=f"ysb{b}_{q}", tag="ysb")
                    for t in range(CPB):
                        j0 = (q * CPB + t) * RPC
                        rhs = qtb[0:C_IN, j0:j0 + RPC, :]
                        ya = ypap.tile([128, RPC, H], F32, name="ya", tag="ya")
                        nc.tensor.matmul(ya[:], wpc_sb[:, 0:128], rhs,
                                         start=True, stop=True)
                        yb = ypbp.tile([64, RPC, H], F32, name="yb", tag="yb")
                        nc.tensor.matmul(yb[:], wpc_sb[:, 128:192], rhs,
                                         start=True, stop=True)
                        for ys, yp in ((ysa, ya), (ysb, yb)):
                            dst = ys[:, RPC * t:RPC * (t + 1), :]
                            if copy_ctr % 2 == 0:
                                nc.scalar.copy(dst, yp[:])
                            else:
                                nc.vector.tensor_copy(dst, yp[:])
                            copy_ctr += 1
                    r0 = q * CPB * RPC
                    nc.sync.dma_start(
                        y_d[b, 0:128, r0:r0 + CPB * RPC, :], ysa[:])
                    nc.sync.dma_start(
                        y_d[b, 128:192, r0:r0 + CPB * RPC, :], ysb[:])

            for b in range(B_PER):
                emit_dw(b)
                if b > 0:
                    emit_pw(b - 1)
            emit_pw(B_PER - 1)

    nc.compile()
    return nc


def _prep_inputs(x, w_pc, w_dc):
    x = np.asarray(x, dtype=np.float32)
    k3 = np.asarray(w_dc, dtype=np.float32).reshape(3, 3)
    Wm = np.asarray(w_pc, dtype=np.float32).reshape(C_OUT, C_IN)

    # [b, h, 114 w, ci]: transpose + W-pad, ci inner, bf16
    xp = np.zeros((B, H, WP, C_IN), dtype=np.float32)
    xp[:, :, 1:1 + W, :] = x.transpose(0, 2, 3, 1)
    xp = xp.astype(ml_dtypes.bfloat16)

    # band[h, v, i] = k3[h - i + 1, v], i-padded to 128
    band = np.zeros((H, 3, IPAD), dtype=np.float32)
    hh, ii = np.meshgrid(np.arange(H), np.arange(H), indexing="ij")
    u = hh - ii + 1
    m = (u >= 0) & (u < 3)
    for v in range(3):
        bv = np.zeros((H, H), dtype=np.float32)
        bv[m] = k3[u[m], v]
        band[:, v, :H] = bv
    band = band.astype(ml_dtypes.bfloat16)

    wpcT = np.ascontiguousarray(Wm.T).astype(ml_dtypes.bfloat16)
    return xp, band, wpcT


def kernel(x, w_pc, w_dc, _trace=False):
    global _NC, LAST_RESULTS
    if _NC is None:
        _NC = _build()

    xp, band, wpcT = _prep_inputs(x, w_pc, w_dc)
    in_maps = [
        {"x": np.ascontiguousarray(xp[i * B_PER:(i + 1) * B_PER]),
         "band": band, "wpcT": wpcT}
        for i in range(N_CORES)
    ]
    res = run_bass_kernel_spmd(_NC, in_maps, list(range(N_CORES)),
                               trace=_trace)
    LAST_RESULTS = res
    # y stored [b, co, j, i] on device -> [b, co, i, j]
    y = np.concatenate([res.results[i]["y"] for i in range(N_CORES)], axis=0)
    y = np.asarray(y, dtype=np.float32).transpose(0, 1, 3, 2)
    return np.ascontiguousarray(y)
